# revision 50
# baseline (speedup 1.0000x reference)
"""CapsLayer kernel: j-sharded, W-stationary mixed bf16/fp8 contraction.

Math: the reference's routing loop is dead (softmax over a size-1 axis is
identically 1), so the output is
    s[b, j, l] = sum_{i,k} W[i, j, l, k] * inputs[b, i, k]
    vj = squash(s, axis=l)  ->  [B, 1, NUM_CAPS, DIM_CAPS]

Sharding: W splits over NUM_CAPS j (4 capsules / 128 output columns per
core); inputs are replicated.  No collectives (an 8-core ncfw
ReduceScatter measures ~42 us of fixed latency).

Precision: the DMA stream is the wall (the fp32 version already ran at
the ~360 B/ns bus limit), so bytes are the whole game: x streams bf16,
W streams bf16 for 7 of 16 k's and fp8e4m3 for the other 9 (8.3 MB/core
vs 21.2 fp32).  PSUM accumulation and the squash run in fp32.  Measured
rel err 1.83e-2 vs the 2e-2 gate, deterministic: it comes entirely from
the host-side quantization in _in_maps, and hardware accumulation
matched the numpy simulation of it exactly (bf16-only variant: 2.1e-3;
10-of-16 fp8 would be 1.93e-2 -- too close to the gate).

PE orientation: W[128i, 128(j,l)] is the STATIONARY operand — full 128
columns enables fast-weight-load — and x[128i, 32b] streams as moving
data.  All 256 matmuls (bf16 and fp8 stationaries per the QK k-split)
accumulate one PSUM chain s_T[(j,l), b]; the transposed result costs 4 DVE
stream-transposes in the epilogue (cheaper than the v4-era identity-
matmul merge + PSUM copy, and the moving bus carries 4x fewer bytes).

DMA: 18 transfers — one per 128-row i-tile, with the last tile split
half/quarter/quarter so the final PE chunk lands a quarter-tile after
the stream ends, and tile 0 issued from the scalar engine's HWDGE queue
(it clears the framework preamble around when sync does, and a second
descriptor stream primes the 16 DMA engines; full sync/scalar
alternation measured slower).  Rows are quarter-packed per QK as
[x bf16 | W bf16 slots | W fp8 slots] so every piece is one contiguous
~1 KB descriptor per partition, plus a 64 B pad to keep the DRAM row
stride off a 4 KiB boundary; matmul operands are bitcast views into the
byte buffer.  Both output halves DMA out concurrently from the two
HWDGE queues as soon as their final multiply lands.

Raw Bass: this walrus build rejects instructions carrying 2+ sem waits, so
all sync is standalone wait_ge ops.  DVE/ACT same-engine RAW needs explicit
semaphores (the pipelines do not interlock through SBUF).
"""

from contextlib import ExitStack

import numpy as np

B = 32
IN_CAPS = 2048
IN_DIM = 16
NUM_CAPS = 32
DIM_CAPS = 32
NCORES = 8
JPC = NUM_CAPS // NCORES          # 4 capsules per core
NJL = JPC * DIM_CAPS              # 128 output columns per core
P = 128
NTILES = IN_CAPS // P             # 16
KQ = IN_DIM // 4                  # 4 k's per quarter-row
XB = KQ * B * 2                   # 256 B of x per quarter-row (k-major, bf16)
# per-quarter k assignment: (bf16 k's, fp8 k's).  9 of 16 k's in fp8:
# measured rel err 1.83e-2 (8/16: 1.72e-2, 10/16: 1.93e-2 -- too close
# to the 2e-2 gate).
QK = [([0, 2], [1, 3]), ([4, 6], [5, 7]), ([8, 10], [9, 11]),
      ([12], [13, 14, 15])]
QBS = [XB + len(b) * NJL * 2 + len(f) * NJL for b, f in QK]  # quarter bytes
QOFF = [sum(QBS[:q]) for q in range(5)]                      # quarter offsets
PAD = 64                          # keep DRAM row stride off 4 KiB
RB = QOFF[4] + PAD                # 4032 B per row

_CACHE = {}


def _build():
    import concourse.bass as bass
    from concourse import mybir

    f32 = mybir.dt.float32
    bf16 = mybir.dt.bfloat16
    f8 = mybir.dt.float8e4
    u8 = mybir.dt.uint8
    nc = bass.Bass()
    xw = nc.declare_dram_parameter("xw", [IN_CAPS, RB], u8, isOutput=False)
    out = nc.declare_dram_parameter("out", [B, NJL], f32, isOutput=True)

    with ExitStack() as ctx:
        xw_sb = ctx.enter_context(nc.sbuf_tensor([P, NTILES * RB], u8))
        sv = ctx.enter_context(nc.sbuf_tensor([B, NJL], f32))
        sq = ctx.enter_context(nc.sbuf_tensor([B, NJL], f32))
        ss = ctx.enter_context(nc.sbuf_tensor([B, JPC], f32))
        rt = ctx.enter_context(nc.sbuf_tensor([B, JPC], f32))
        rc = ctx.enter_context(nc.sbuf_tensor([B, JPC], f32))
        sm = ctx.enter_context(nc.sbuf_tensor([B, JPC], f32))
        fsc = ctx.enter_context(nc.sbuf_tensor([B, JPC], f32))
        warm = ctx.enter_context(nc.sbuf_tensor([B, 1], f32))
        vout = ctx.enter_context(nc.sbuf_tensor([B, NJL], f32))
        psT = ctx.enter_context(nc.psum_tensor([P, B], f32))

        NDMA = NTILES + 2         # 15 whole tiles + half + quarter + quarter
        tsem = [ctx.enter_context(nc.semaphore(f"t{t}")) for t in range(NDMA)]
        pe_sem = ctx.enter_context(nc.semaphore("pe"))
        act_sem = ctx.enter_context(nc.semaphore("act"))
        dve_sem = ctx.enter_context(nc.semaphore("dve"))
        odma = ctx.enter_context(nc.semaphore("odma"))
        block = ctx.enter_context(nc.Block())

        lt = NTILES - 1
        # last-tile pieces as (start_byte, end_byte) within the row
        pieces = [(0, QOFF[2]), (QOFF[2], QOFF[3]), (QOFF[3], RB)]

        @block.sync
        def _(sync):
            # tile 0 is issued by the scalar engine's HWDGE queue: it clears
            # the framework preamble around when sync does, and a second
            # descriptor stream keeps the 16 DMA engines better fed at the
            # start.  (Tried and rejected: full sync/scalar alternation, and
            # gpsimd SWDGE pre-streaming -- gpsimd's preamble is just as
            # long and its Q7 descriptor generation adds ~3 us.)
            # sync carries exactly 16 xw entries (tiles 1-13 + 3 last-tile
            # pieces): a 17th would overflow the 16-deep HWDGE ring and
            # stall the final piece's issue ~1.5 us (measured)
            for t in range(1, lt - 1):
                sync.dma_start(
                    out=xw_sb[:, t * RB:(t + 1) * RB],
                    in_=xw[t * P:(t + 1) * P, :],
                ).then_inc(tsem[t], 16)
            for p, (lo, hi) in enumerate(pieces):
                sync.dma_start(
                    out=xw_sb[:, lt * RB + lo:lt * RB + hi],
                    in_=xw[lt * P:(lt + 1) * P, lo:hi],
                ).then_inc(tsem[lt + p], 16)
            sync.wait_ge(dve_sem, 7)
            sync.dma_start(
                out=out[:, :NJL // 2], in_=vout[:, :NJL // 2]
            ).then_inc(odma, 16)
            sync.wait_ge(odma, 32)

        @block.tensor
        def _(tensor):
            last = NTILES * IN_DIM - 1
            n = 0
            for t in range(NTILES):
                for q in range(4):
                    if t < lt:
                        if q == 0:
                            tensor.wait_ge(tsem[t], 16)
                    elif q == 0:
                        tensor.wait_ge(tsem[lt], 16)
                    elif q >= 2:
                        tensor.wait_ge(tsem[lt + q - 1], 16)
                    base = t * RB + QOFF[q]
                    bks, fks = QK[q]
                    wbase = base + XB
                    for w, (ks, dt, wb) in enumerate(
                        [(bks, bf16, 2), (fks, f8, 1)]
                    ):
                        for kk in ks:
                            mm = nc.tensor.matmul(
                                psT[:, :],
                                xw_sb[:, wbase:wbase + NJL * wb].bitcast(dt),
                                xw_sb[:, base + (kk - 4 * q) * 64:
                                      base + (kk - 4 * q + 1) * 64].bitcast(bf16),
                                start=(n == 0),
                                stop=(n == last),
                            )
                            wbase += NJL * wb
                            n += 1
            mm.then_inc(pe_sem, 1)

        @block.vector
        def _(vector):
            vector.wait_ge(pe_sem, 1)
            # s_T[(j,l), b] -> sv[b, (j,l)] via 4 32x32 stream transposes
            for g in range(JPC):
                tr = nc.vector.transpose(
                    out=sv[:, g * DIM_CAPS:(g + 1) * DIM_CAPS],
                    in_=psT[g * DIM_CAPS:(g + 1) * DIM_CAPS, :],
                )
            tr.then_inc(dve_sem, 1)
            vector.wait_ge(dve_sem, 1)
            nc.vector.tensor_mul(sq[:, :], sv[:, :], sv[:, :]).then_inc(dve_sem, 1)
            vector.wait_ge(dve_sem, 2)
            nc.vector.reduce_sum(
                out=ss[:, :],
                in_=sq[:, :].rearrange("p (g d) -> p g d", g=JPC),
                axis=mybir.AxisListType.X,
            ).then_inc(dve_sem, 1)
            # squash scale: fsc = ss/((1+ss)*sqrt(ss+eps)) = sqrt(ss)/(1+ss)
            # (eps is 1e-12-relative at this data's ss ~ 4e4, so it drops,
            # letting ACT's sqrt run concurrently with the 1+ss -> recip
            # path here instead of feeding a serial chain)
            vector.wait_ge(dve_sem, 3)
            nc.vector.tensor_scalar_add(sm[:, :], ss[:, :], 1.0).then_inc(
                dve_sem, 1
            )
            vector.wait_ge(dve_sem, 4)
            nc.vector.reciprocal(out=rc[:, :], in_=sm[:, :]).then_inc(dve_sem, 1)
            vector.wait_ge(act_sem, 1)
            vector.wait_ge(dve_sem, 5)
            nc.vector.tensor_mul(fsc[:, :], rt[:, :], rc[:, :]).then_inc(dve_sem, 1)
            vector.wait_ge(dve_sem, 6)
            half = JPC // 2
            nc.vector.tensor_mul(
                vout[:, :NJL // 2].rearrange("p (g d) -> p g d", g=half),
                sv[:, :NJL // 2].rearrange("p (g d) -> p g d", g=half),
                fsc[:, :half, None].broadcast_to((B, half, DIM_CAPS)),
            ).then_inc(dve_sem, 1)
            vector.wait_ge(dve_sem, 7)
            nc.vector.tensor_mul(
                vout[:, NJL // 2:].rearrange("p (g d) -> p g d", g=half),
                sv[:, NJL // 2:].rearrange("p (g d) -> p g d", g=half),
                fsc[:, half:, None].broadcast_to((B, half, DIM_CAPS)),
            ).then_inc(dve_sem, 1)

        @block.scalar
        def _(scalar):
            for t in (0, lt - 1):
                nc.scalar.dma_start(
                    out=xw_sb[:, t * RB:(t + 1) * RB],
                    in_=xw[t * P:(t + 1) * P, :],
                ).then_inc(tsem[t], 16)
            # dummy Sqrt pulls the ~1.3us ACT table load off the epilogue
            # critical path (operands are a scratch tile nobody else touches)
            nc.scalar.activation(
                out=warm[:, :], in_=warm[:, :],
                func=mybir.ActivationFunctionType.Sqrt, bias=warm[:, :],
            )
            scalar.wait_ge(dve_sem, 3)
            nc.scalar.activation(
                out=rt[:, :], in_=ss[:, :],
                func=mybir.ActivationFunctionType.Sqrt, bias=0.0,
            ).then_inc(act_sem, 1)
            # second output half issued here, in parallel with sync's first
            scalar.wait_ge(dve_sem, 8)
            nc.scalar.dma_start(
                out=out[:, NJL // 2:], in_=vout[:, NJL // 2:]
            ).then_inc(odma, 16)

    return nc


def _in_maps(inputs, W):
    import ml_dtypes

    bf = np.dtype(ml_dtypes.bfloat16)
    f8 = np.dtype(ml_dtypes.float8_e4m3)
    # x packed k-major per i-row: [i, k, b]
    x_t = np.ascontiguousarray(np.transpose(inputs, (1, 2, 0))).astype(bf)
    maps = []
    for c in range(NCORES):
        # W slice -> [i, k, (j, l)] k-major per i-row
        Wc = np.ascontiguousarray(
            np.transpose(W[:, c * JPC:(c + 1) * JPC], (0, 3, 1, 2))
        ).reshape(IN_CAPS, IN_DIM, NJL)
        xwc = np.zeros((IN_CAPS, RB), dtype=np.uint8)
        for q in range(4):
            o = QOFF[q]
            xq = np.ascontiguousarray(
                x_t[:, q * KQ:(q + 1) * KQ]
            ).reshape(IN_CAPS, KQ * B)
            xwc[:, o:o + XB] = xq.view(np.uint8)
            o += XB
            bks, fks = QK[q]
            for ks, dt in [(bks, bf), (fks, f8)]:
                for kk in ks:
                    wk = np.ascontiguousarray(Wc[:, kk]).astype(dt)
                    nb = NJL * dt.itemsize
                    xwc[:, o:o + nb] = wk.view(np.uint8)
                    o += nb
        maps.append({"xw": xwc})
    return maps


def kernel(inputs, W):
    from concourse.bass_utils import run_bass_kernel_spmd

    inputs = np.asarray(inputs, dtype=np.float32)
    W = np.asarray(W, dtype=np.float32)
    if "nc" not in _CACHE:
        _CACHE["nc"] = _build()
    res = run_bass_kernel_spmd(_CACHE["nc"], _in_maps(inputs, W), list(range(NCORES)))
    return np.concatenate(
        [res.results[c]["out"].reshape(B, 1, JPC, DIM_CAPS) for c in range(NCORES)],
        axis=2,
    )


# revision 56
# speedup vs baseline: 1.0083x; 1.0083x over previous
"""CapsLayer kernel: j-sharded, W-stationary mixed bf16/fp8 contraction.

Math: the reference's routing loop is dead (softmax over a size-1 axis is
identically 1), so the output is
    s[b, j, l] = sum_{i,k} W[i, j, l, k] * inputs[b, i, k]
    vj = squash(s, axis=l)  ->  [B, 1, NUM_CAPS, DIM_CAPS]

Sharding: W splits over NUM_CAPS j (4 capsules / 128 output columns per
core); inputs are replicated.  No collectives (an 8-core ncfw
ReduceScatter measures ~42 us of fixed latency).

Precision: the DMA stream is the wall (the fp32 version already ran at
the ~360 B/ns bus limit), so bytes are the whole game: x streams bf16,
W streams bf16 for 7 of 16 k's and fp8e4m3 for the other 9 (8.3 MB/core
vs 21.2 fp32).  PSUM accumulation and the squash run in fp32.  Measured
rel err 1.83e-2 vs the 2e-2 gate, deterministic: it comes entirely from
the host-side quantization in _in_maps, and hardware accumulation
matched the numpy simulation of it exactly (bf16-only variant: 2.1e-3;
10-of-16 fp8 would be 1.93e-2 -- too close to the gate).

PE orientation: W[128i, 128(j,l)] is the STATIONARY operand — full 128
columns enables fast-weight-load — and x[128i, 32b] streams as moving
data.  All 256 matmuls (bf16 and fp8 stationaries per the QK k-split)
accumulate one PSUM chain s_T[(j,l), b]; the transposed result costs 4 DVE
stream-transposes in the epilogue (cheaper than the v4-era identity-
matmul merge + PSUM copy, and the moving bus carries 4x fewer bytes).

DMA: 18 transfers — one per 128-row i-tile, with the last tile split
half/quarter/quarter so the final PE chunk lands a quarter-tile after
the stream ends, and tile 0 issued from the scalar engine's HWDGE queue
(it clears the framework preamble around when sync does, and a second
descriptor stream primes the 16 DMA engines; full sync/scalar
alternation measured slower).  Rows are quarter-packed per QK as
[x bf16 | W bf16 slots | W fp8 slots] so every piece is one contiguous
~1 KB descriptor per partition, plus a 64 B pad to keep the DRAM row
stride off a 4 KiB boundary; matmul operands are bitcast views into the
byte buffer.  Both output halves DMA out concurrently from the two
HWDGE queues as soon as their final multiply lands.

Raw Bass: this walrus build rejects instructions carrying 2+ sem waits, so
all sync is standalone wait_ge ops.  DVE/ACT same-engine RAW needs explicit
semaphores (the pipelines do not interlock through SBUF).
"""

from contextlib import ExitStack

import numpy as np

B = 32
IN_CAPS = 2048
IN_DIM = 16
NUM_CAPS = 32
DIM_CAPS = 32
NCORES = 8
JPC = NUM_CAPS // NCORES          # 4 capsules per core
NJL = JPC * DIM_CAPS              # 128 output columns per core
P = 128
NTILES = IN_CAPS // P             # 16
KQ = IN_DIM // 4                  # 4 k's per quarter-row
XB = KQ * B * 2                   # 256 B of x per quarter-row (k-major, bf16)
# per-quarter k assignment: (bf16 k's, fp8 k's).  9 of 16 k's in fp8:
# measured rel err 1.83e-2 (8/16: 1.72e-2, 10/16: 1.93e-2 -- too close
# to the 2e-2 gate).
QK = [([0, 2], [1, 3]), ([4, 6], [5, 7]), ([8, 10], [9, 11]),
      ([12], [13, 14, 15])]
QBS = [XB + len(b) * NJL * 2 + len(f) * NJL for b, f in QK]  # quarter bytes
QOFF = [sum(QBS[:q]) for q in range(5)]                      # quarter offsets
PAD = 64                          # keep DRAM row stride off 4 KiB
RB = QOFF[4] + PAD                # 4032 B per row

_CACHE = {}


def _build():
    import concourse.bass as bass
    from concourse import mybir

    f32 = mybir.dt.float32
    bf16 = mybir.dt.bfloat16
    f8 = mybir.dt.float8e4
    u8 = mybir.dt.uint8
    nc = bass.Bass()
    xw = nc.declare_dram_parameter("xw", [IN_CAPS, RB], u8, isOutput=False)
    out = nc.declare_dram_parameter("out", [B, NJL], f32, isOutput=True)

    with ExitStack() as ctx:
        xw_sb = ctx.enter_context(nc.sbuf_tensor([P, NTILES * RB], u8))
        sv = ctx.enter_context(nc.sbuf_tensor([B, NJL], f32))
        sq = ctx.enter_context(nc.sbuf_tensor([B, NJL], f32))
        ss = ctx.enter_context(nc.sbuf_tensor([B, JPC], f32))
        rt = ctx.enter_context(nc.sbuf_tensor([B, JPC], f32))
        rc = ctx.enter_context(nc.sbuf_tensor([B, JPC], f32))
        sm = ctx.enter_context(nc.sbuf_tensor([B, JPC], f32))
        fsc = ctx.enter_context(nc.sbuf_tensor([B, JPC], f32))
        warm = ctx.enter_context(nc.sbuf_tensor([B, 1], f32))
        vout = ctx.enter_context(nc.sbuf_tensor([B, NJL], f32))
        psT = ctx.enter_context(nc.psum_tensor([P, B], f32))

        NDMA = NTILES + 2         # 15 whole tiles + half + quarter + quarter
        tsem = [ctx.enter_context(nc.semaphore(f"t{t}")) for t in range(NDMA)]
        pe_sem = ctx.enter_context(nc.semaphore("pe"))
        act_sem = ctx.enter_context(nc.semaphore("act"))
        dve_sem = ctx.enter_context(nc.semaphore("dve"))
        odma = ctx.enter_context(nc.semaphore("odma"))
        block = ctx.enter_context(nc.Block())

        lt = NTILES - 1
        # last-tile pieces as (start_byte, end_byte) within the row
        pieces = [(0, QOFF[2]), (QOFF[2], QOFF[3]), (QOFF[3], RB)]

        @block.sync
        def _(sync):
            # tile 0 is issued by the scalar engine's HWDGE queue: it clears
            # the framework preamble around when sync does, and a second
            # descriptor stream keeps the 16 DMA engines better fed at the
            # start.  (Tried and rejected: full sync/scalar alternation, and
            # gpsimd SWDGE pre-streaming -- gpsimd's preamble is just as
            # long and its Q7 descriptor generation adds ~3 us.)
            # sync carries exactly 16 xw entries (tiles 1-13 + 3 last-tile
            # pieces): a 17th would overflow the 16-deep HWDGE ring and
            # stall the final piece's issue ~1.5 us (measured)
            for t in range(1, lt - 1):
                sync.dma_start(
                    out=xw_sb[:, t * RB:(t + 1) * RB],
                    in_=xw[t * P:(t + 1) * P, :],
                ).then_inc(tsem[t], 16)
            for p, (lo, hi) in enumerate(pieces):
                sync.dma_start(
                    out=xw_sb[:, lt * RB + lo:lt * RB + hi],
                    in_=xw[lt * P:(lt + 1) * P, lo:hi],
                ).then_inc(tsem[lt + p], 16)
            sync.wait_ge(dve_sem, 7)
            sync.dma_start(
                out=out[:, :NJL // 2], in_=vout[:, :NJL // 2]
            ).then_inc(odma, 16)
            sync.wait_ge(odma, 32)

        @block.tensor
        def _(tensor):
            last = NTILES * IN_DIM - 1
            n = 0
            for t in range(NTILES):
                for q in range(4):
                    if t < lt:
                        if q == 0:
                            tensor.wait_ge(tsem[t], 16)
                    elif q == 0:
                        tensor.wait_ge(tsem[lt], 16)
                    elif q >= 2:
                        tensor.wait_ge(tsem[lt + q - 1], 16)
                    base = t * RB + QOFF[q]
                    bks, fks = QK[q]
                    wbase = base + XB
                    for w, (ks, dt, wb) in enumerate(
                        [(bks, bf16, 2), (fks, f8, 1)]
                    ):
                        for kk in ks:
                            mm = nc.tensor.matmul(
                                psT[:, :],
                                xw_sb[:, wbase:wbase + NJL * wb].bitcast(dt),
                                xw_sb[:, base + (kk - 4 * q) * 64:
                                      base + (kk - 4 * q + 1) * 64].bitcast(bf16),
                                start=(n == 0),
                                stop=(n == last),
                            )
                            wbase += NJL * wb
                            n += 1
            mm.then_inc(pe_sem, 1)

        @block.vector
        def _(vector):
            vector.wait_ge(pe_sem, 1)
            # s_T[(j,l), b] -> sv[b, (j,l)] via 4 32x32 stream transposes
            for g in range(JPC):
                tr = nc.vector.transpose(
                    out=sv[:, g * DIM_CAPS:(g + 1) * DIM_CAPS],
                    in_=psT[g * DIM_CAPS:(g + 1) * DIM_CAPS, :],
                )
            tr.then_inc(dve_sem, 1)
            vector.wait_ge(dve_sem, 1)
            nc.vector.tensor_mul(sq[:, :], sv[:, :], sv[:, :]).then_inc(dve_sem, 1)
            vector.wait_ge(dve_sem, 2)
            nc.vector.reduce_sum(
                out=ss[:, :],
                in_=sq[:, :].rearrange("p (g d) -> p g d", g=JPC),
                axis=mybir.AxisListType.X,
            ).then_inc(dve_sem, 1)
            # squash scale: fsc = ss/((1+ss)*sqrt(ss+eps)) = sqrt(ss)/(1+ss)
            # (eps is 1e-12-relative at this data's ss ~ 4e4, so it drops,
            # letting ACT's sqrt run concurrently with the 1+ss -> recip
            # path here instead of feeding a serial chain)
            vector.wait_ge(dve_sem, 3)
            nc.vector.tensor_scalar_add(sm[:, :], ss[:, :], 1.0).then_inc(
                dve_sem, 1
            )
            vector.wait_ge(dve_sem, 4)
            nc.vector.reciprocal(out=rc[:, :], in_=sm[:, :]).then_inc(dve_sem, 1)
            vector.wait_ge(act_sem, 1)
            vector.wait_ge(dve_sem, 5)
            nc.vector.tensor_mul(fsc[:, :], rt[:, :], rc[:, :]).then_inc(dve_sem, 1)
            vector.wait_ge(dve_sem, 6)
            half = JPC // 2
            nc.vector.tensor_mul(
                vout[:, :NJL // 2].rearrange("p (g d) -> p g d", g=half),
                sv[:, :NJL // 2].rearrange("p (g d) -> p g d", g=half),
                fsc[:, :half, None].broadcast_to((B, half, DIM_CAPS)),
            ).then_inc(dve_sem, 1)
            vector.wait_ge(dve_sem, 7)
            nc.vector.tensor_mul(
                vout[:, NJL // 2:].rearrange("p (g d) -> p g d", g=half),
                sv[:, NJL // 2:].rearrange("p (g d) -> p g d", g=half),
                fsc[:, half:, None].broadcast_to((B, half, DIM_CAPS)),
            ).then_inc(dve_sem, 1)

        @block.scalar
        def _(scalar):
            for t in (0, lt - 1):
                nc.scalar.dma_start(
                    out=xw_sb[:, t * RB:(t + 1) * RB],
                    in_=xw[t * P:(t + 1) * P, :],
                ).then_inc(tsem[t], 16)
            # dummy Sqrt pulls the ~1.3us ACT table load off the epilogue
            # critical path (operands are a scratch tile nobody else touches)
            nc.scalar.activation(
                out=warm[:, :], in_=warm[:, :],
                func=mybir.ActivationFunctionType.Sqrt, bias=warm[:, :],
            )
            scalar.wait_ge(dve_sem, 3)
            nc.scalar.activation(
                out=rt[:, :], in_=ss[:, :],
                func=mybir.ActivationFunctionType.Sqrt, bias=0.0,
            ).then_inc(act_sem, 1)
            # second output half issued here, in parallel with sync's first
            scalar.wait_ge(dve_sem, 8)
            nc.scalar.dma_start(
                out=out[:, NJL // 2:], in_=vout[:, NJL // 2:]
            ).then_inc(odma, 16)

    return nc


def _in_maps(inputs, W):
    import ml_dtypes

    bf = np.dtype(ml_dtypes.bfloat16)
    f8 = np.dtype(ml_dtypes.float8_e4m3)
    # x packed k-major per i-row: [i, k, b]
    x_t = np.ascontiguousarray(np.transpose(inputs, (1, 2, 0))).astype(bf)
    maps = []
    for c in range(NCORES):
        # W slice -> [i, k, (j, l)] k-major per i-row
        Wc = np.ascontiguousarray(
            np.transpose(W[:, c * JPC:(c + 1) * JPC], (0, 3, 1, 2))
        ).reshape(IN_CAPS, IN_DIM, NJL)
        xwc = np.zeros((IN_CAPS, RB), dtype=np.uint8)
        for q in range(4):
            o = QOFF[q]
            xq = np.ascontiguousarray(
                x_t[:, q * KQ:(q + 1) * KQ]
            ).reshape(IN_CAPS, KQ * B)
            xwc[:, o:o + XB] = xq.view(np.uint8)
            o += XB
            bks, fks = QK[q]
            for ks, dt in [(bks, bf), (fks, f8)]:
                for kk in ks:
                    wk = np.ascontiguousarray(Wc[:, kk]).astype(dt)
                    nb = NJL * dt.itemsize
                    xwc[:, o:o + nb] = wk.view(np.uint8)
                    o += nb
        maps.append({"xw": xwc})
    return maps


def kernel(inputs, W):
    from concourse.bass_utils import run_bass_kernel_spmd

    inputs = np.asarray(inputs, dtype=np.float32)
    W = np.asarray(W, dtype=np.float32)
    if "nc" not in _CACHE:
        _CACHE["nc"] = _build()
    res = run_bass_kernel_spmd(_CACHE["nc"], _in_maps(inputs, W), list(range(NCORES)))
    return np.concatenate(
        [res.results[c]["out"].reshape(B, 1, JPC, DIM_CAPS) for c in range(NCORES)],
        axis=2,
    )


# revision 63
# speedup vs baseline: 1.1295x; 1.1202x over previous
"""CapsLayer kernel: j-sharded, W-stationary mixed bf16/fp8 contraction.

Math: the reference's routing loop is dead (softmax over a size-1 axis is
identically 1), so the output is
    s[b, j, l] = sum_{i,k} W[i, j, l, k] * inputs[b, i, k]
    vj = squash(s, axis=l)  ->  [B, 1, NUM_CAPS, DIM_CAPS]

Sharding: W splits over NUM_CAPS j (4 capsules / 128 output columns per
core); inputs are replicated.  No collectives (an 8-core ncfw
ReduceScatter measures ~42 us of fixed latency).

Precision: the DMA stream is the wall (the fp32 version already ran at
the ~360 B/ns bus limit), so bytes are the whole game: x streams bf16,
W streams bf16 for 7 of 16 k's and fp8e4m3 for the other 9 (8.3 MB/core
vs 21.2 fp32).  PSUM accumulation and the squash run in fp32.  Measured
rel err 1.83e-2 vs the 2e-2 gate, deterministic: it comes entirely from
the host-side quantization in _in_maps, and hardware accumulation
matched the numpy simulation of it exactly (bf16-only variant: 2.1e-3;
10-of-16 fp8 would be 1.93e-2 -- too close to the gate).

PE orientation: W[128i, 128(j,l)] is the STATIONARY operand — full 128
columns enables fast-weight-load — and x[128i, 32b] streams as moving
data.  All 256 matmuls (bf16 and fp8 stationaries per the QK k-split)
accumulate one PSUM chain s_T[(j,l), b]; the transposed result costs 4 DVE
stream-transposes in the epilogue (cheaper than the v4-era identity-
matmul merge + PSUM copy, and the moving bus carries 4x fewer bytes).

DMA: 18 transfers — one per 128-row i-tile, with the last tile split
half/quarter/quarter so the final PE chunk lands a quarter-tile after
the stream ends, and tile 0 issued from the scalar engine's HWDGE queue
(it clears the framework preamble around when sync does, and a second
descriptor stream primes the 16 DMA engines; full sync/scalar
alternation measured slower).  Rows are quarter-packed per QK as
[x bf16 | W bf16 slots | W fp8 slots] so every piece is one contiguous
~1 KB descriptor per partition, plus a 64 B pad to keep the DRAM row
stride off a 4 KiB boundary; matmul operands are bitcast views into the
byte buffer.  Both output halves DMA out concurrently from the two
HWDGE queues as soon as their final multiply lands.

Raw Bass: this walrus build rejects instructions carrying 2+ sem waits, so
all sync is standalone wait_ge ops.  DVE/ACT same-engine RAW needs explicit
semaphores (the pipelines do not interlock through SBUF).
"""

from contextlib import ExitStack

import numpy as np

B = 32
IN_CAPS = 2048
IN_DIM = 16
NUM_CAPS = 32
DIM_CAPS = 32
NCORES = 8
JPC = NUM_CAPS // NCORES          # 4 capsules per core
NJL = JPC * DIM_CAPS              # 128 output columns per core
P = 128
NTILES = IN_CAPS // P             # 16
KQ = IN_DIM // 4                  # 4 k's per quarter-row
XB = KQ * B * 2                   # 256 B of x per quarter-row (k-major, bf16)
# per-quarter k assignment: (bf16 k's, fp8 k's).  9 of 16 k's in fp8:
# measured rel err 1.83e-2 (8/16: 1.72e-2, 10/16: 1.93e-2 -- too close
# to the 2e-2 gate).
QK = [([0, 2], [1, 3]), ([4, 6], [5, 7]), ([8, 10], [9, 11]),
      ([12], [13, 14, 15])]
QBS = [XB + len(b) * NJL * 2 + len(f) * NJL for b, f in QK]  # quarter bytes
QOFF = [sum(QBS[:q]) for q in range(5)]                      # quarter offsets
PAD = 64                          # keep DRAM row stride off 4 KiB
RB = QOFF[4] + PAD                # 4032 B per row

_CACHE = {}


def _build():
    import concourse.bass as bass
    from concourse import mybir

    f32 = mybir.dt.float32
    bf16 = mybir.dt.bfloat16
    f8 = mybir.dt.float8e4
    u8 = mybir.dt.uint8
    nc = bass.Bass()
    xw = nc.declare_dram_parameter("xw", [IN_CAPS, RB], u8, isOutput=False)
    # output stays in the epilogue's block layout (partition 32g+b holds
    # capsule g of batch b); the host unshard step untangles it for free
    out = nc.declare_dram_parameter("out", [P, DIM_CAPS], f32, isOutput=True)

    with ExitStack() as ctx:
        xw_sb = ctx.enter_context(nc.sbuf_tensor([P, NTILES * RB], u8))
        sv = ctx.enter_context(nc.sbuf_tensor([P, DIM_CAPS], f32))
        sq = ctx.enter_context(nc.sbuf_tensor([P, DIM_CAPS], f32))
        ss = ctx.enter_context(nc.sbuf_tensor([P, 1], f32))
        rt = ctx.enter_context(nc.sbuf_tensor([P, 1], f32))
        rc = ctx.enter_context(nc.sbuf_tensor([P, 1], f32))
        sm = ctx.enter_context(nc.sbuf_tensor([P, 1], f32))
        fsc = ctx.enter_context(nc.sbuf_tensor([P, 1], f32))
        warm = ctx.enter_context(nc.sbuf_tensor([B, 1], f32))
        vout = ctx.enter_context(nc.sbuf_tensor([P, DIM_CAPS], f32))
        psT = ctx.enter_context(nc.psum_tensor([P, B], f32))

        NDMA = NTILES + 2         # 15 whole tiles + half + quarter + quarter
        tsem = [ctx.enter_context(nc.semaphore(f"t{t}")) for t in range(NDMA)]
        pe_sem = ctx.enter_context(nc.semaphore("pe"))
        act_sem = ctx.enter_context(nc.semaphore("act"))
        dve_sem = ctx.enter_context(nc.semaphore("dve"))
        odma = ctx.enter_context(nc.semaphore("odma"))
        block = ctx.enter_context(nc.Block())

        lt = NTILES - 1
        # last-tile pieces as (start_byte, end_byte) within the row
        pieces = [(0, QOFF[2]), (QOFF[2], QOFF[3]), (QOFF[3], RB)]

        @block.sync
        def _(sync):
            # tile 0 is issued by the scalar engine's HWDGE queue: it clears
            # the framework preamble around when sync does, and a second
            # descriptor stream keeps the 16 DMA engines better fed at the
            # start.  (Tried and rejected: full sync/scalar alternation, and
            # gpsimd SWDGE pre-streaming -- gpsimd's preamble is just as
            # long and its Q7 descriptor generation adds ~3 us.)
            # sync carries exactly 16 xw entries (tiles 1-13 + 3 last-tile
            # pieces): a 17th would overflow the 16-deep HWDGE ring and
            # stall the final piece's issue ~1.5 us (measured)
            for t in range(1, lt - 1):
                sync.dma_start(
                    out=xw_sb[:, t * RB:(t + 1) * RB],
                    in_=xw[t * P:(t + 1) * P, :],
                ).then_inc(tsem[t], 16)
            for p, (lo, hi) in enumerate(pieces):
                sync.dma_start(
                    out=xw_sb[:, lt * RB + lo:lt * RB + hi],
                    in_=xw[lt * P:(lt + 1) * P, lo:hi],
                ).then_inc(tsem[lt + p], 16)
            sync.wait_ge(dve_sem, 7)
            sync.dma_start(out=out[:, :], in_=vout[:, :]).then_inc(odma, 16)
            sync.wait_ge(odma, 16)

        @block.tensor
        def _(tensor):
            last = NTILES * IN_DIM - 1
            n = 0
            for t in range(NTILES):
                for q in range(4):
                    if t < lt:
                        if q == 0:
                            tensor.wait_ge(tsem[t], 16)
                    elif q == 0:
                        tensor.wait_ge(tsem[lt], 16)
                    elif q >= 2:
                        tensor.wait_ge(tsem[lt + q - 1], 16)
                    base = t * RB + QOFF[q]
                    bks, fks = QK[q]
                    wbase = base + XB
                    for w, (ks, dt, wb) in enumerate(
                        [(bks, bf16, 2), (fks, f8, 1)]
                    ):
                        for kk in ks:
                            mm = nc.tensor.matmul(
                                psT[:, :],
                                xw_sb[:, wbase:wbase + NJL * wb].bitcast(dt),
                                xw_sb[:, base + (kk - 4 * q) * 64:
                                      base + (kk - 4 * q + 1) * 64].bitcast(bf16),
                                start=(n == 0),
                                stop=(n == last),
                            )
                            wbase += NJL * wb
                            n += 1
            mm.then_inc(pe_sem, 1)

        @block.vector
        def _(vector):
            vector.wait_ge(pe_sem, 1)
            # one-shot block transpose: sv[32g+b, l] = psT[32g+l, b]
            # (StreamTranspose transposes each 32x32 block in place, so the
            # capsule reduce below is a plain per-partition reduce and the
            # squash scale a per-partition scalar)
            nc.vector.transpose(out=sv[:, :], in_=psT[:, :]).then_inc(dve_sem, 1)
            vector.wait_ge(dve_sem, 1)
            nc.vector.tensor_mul(sq[:, :], sv[:, :], sv[:, :]).then_inc(dve_sem, 1)
            vector.wait_ge(dve_sem, 2)
            nc.vector.reduce_sum(
                out=ss[:, :], in_=sq[:, :], axis=mybir.AxisListType.X
            ).then_inc(dve_sem, 1)
            # squash scale: fsc = ss/((1+ss)*sqrt(ss+eps)) = sqrt(ss)/(1+ss)
            # (eps is 1e-12-relative at this data's ss ~ 4e4, so it drops,
            # letting ACT's sqrt run concurrently with the 1+ss -> recip
            # path here instead of feeding a serial chain)
            vector.wait_ge(dve_sem, 3)
            nc.vector.tensor_scalar_add(sm[:, :], ss[:, :], 1.0).then_inc(
                dve_sem, 1
            )
            vector.wait_ge(dve_sem, 4)
            nc.vector.reciprocal(out=rc[:, :], in_=sm[:, :]).then_inc(dve_sem, 1)
            vector.wait_ge(act_sem, 1)
            vector.wait_ge(dve_sem, 5)
            nc.vector.tensor_mul(fsc[:, :], rt[:, :], rc[:, :]).then_inc(dve_sem, 1)
            vector.wait_ge(dve_sem, 6)
            nc.vector.tensor_scalar_mul(
                out=vout[:, :], in0=sv[:, :], scalar1=fsc[:, :]
            ).then_inc(dve_sem, 1)

        @block.scalar
        def _(scalar):
            for t in (0, lt - 1):
                nc.scalar.dma_start(
                    out=xw_sb[:, t * RB:(t + 1) * RB],
                    in_=xw[t * P:(t + 1) * P, :],
                ).then_inc(tsem[t], 16)
            # dummy Sqrt pulls the ~1.3us ACT table load off the epilogue
            # critical path (operands are a scratch tile nobody else touches)
            nc.scalar.activation(
                out=warm[:, :], in_=warm[:, :],
                func=mybir.ActivationFunctionType.Sqrt, bias=warm[:, :],
            )
            scalar.wait_ge(dve_sem, 3)
            nc.scalar.activation(
                out=rt[:, :], in_=ss[:, :],
                func=mybir.ActivationFunctionType.Sqrt, bias=0.0,
            ).then_inc(act_sem, 1)

    return nc


def _in_maps(inputs, W):
    import ml_dtypes

    bf = np.dtype(ml_dtypes.bfloat16)
    f8 = np.dtype(ml_dtypes.float8_e4m3)
    # x packed k-major per i-row: [i, k, b]
    x_t = np.ascontiguousarray(np.transpose(inputs, (1, 2, 0))).astype(bf)
    maps = []
    for c in range(NCORES):
        # W slice -> [i, k, (j, l)] k-major per i-row
        Wc = np.ascontiguousarray(
            np.transpose(W[:, c * JPC:(c + 1) * JPC], (0, 3, 1, 2))
        ).reshape(IN_CAPS, IN_DIM, NJL)
        xwc = np.zeros((IN_CAPS, RB), dtype=np.uint8)
        for q in range(4):
            o = QOFF[q]
            xq = np.ascontiguousarray(
                x_t[:, q * KQ:(q + 1) * KQ]
            ).reshape(IN_CAPS, KQ * B)
            xwc[:, o:o + XB] = xq.view(np.uint8)
            o += XB
            bks, fks = QK[q]
            for ks, dt in [(bks, bf), (fks, f8)]:
                for kk in ks:
                    wk = np.ascontiguousarray(Wc[:, kk]).astype(dt)
                    nb = NJL * dt.itemsize
                    xwc[:, o:o + nb] = wk.view(np.uint8)
                    o += nb
        maps.append({"xw": xwc})
    return maps


def kernel(inputs, W):
    from concourse.bass_utils import run_bass_kernel_spmd

    inputs = np.asarray(inputs, dtype=np.float32)
    W = np.asarray(W, dtype=np.float32)
    if "nc" not in _CACHE:
        _CACHE["nc"] = _build()
    res = run_bass_kernel_spmd(_CACHE["nc"], _in_maps(inputs, W), list(range(NCORES)))
    # device output is block-transposed: row 32g+b = capsule g, batch b
    return np.concatenate(
        [
            res.results[c]["out"]
            .reshape(JPC, B, DIM_CAPS)
            .transpose(1, 0, 2)
            .reshape(B, 1, JPC, DIM_CAPS)
            for c in range(NCORES)
        ],
        axis=2,
    )
